# revision 8
# baseline (speedup 1.0000x reference)
"""Trainium2 Bass kernel for nn_PopulationSNN: 3-layer LIF SNN, T=100 timesteps.

Sharding: data-parallel over batch — 2048 rows split across 8 NeuronCores
(256 rows each); weights replicated; the sequential timestep scan runs
independently per shard.

Per-core structure, software-pipelined across layers (layer L runs one
timestep behind layer L-1):
  phase A : g1[t] = x_t @ (0.5*W1).T + 0.5*b1 on TensorE (bf16 operands,
            fp32 PSUM accumulate; bias via a K=1 ones-matmul).
  LIF     : state u = pre-reset membrane. One fused custom-DVE op:
               u' = select(u < 1, u, 0)*0.5 + g + bias
  spikes  : n = sign(u - 1) on ScalarE as bf16; the next layer's matmul
            consumes n directly with folded weights W'' = 0.25*W and
            bias'' = 0.25*rowsum(W) + 0.5*b   (since s = (n+1)/2).
  output  : acc += n3 on VectorE; host computes (acc + T) / (2T).
"""

from contextlib import ExitStack

import numpy as np
import ml_dtypes

import concourse.bacc as bacc
import concourse.mybir as mybir
import concourse.tile as tile
import concourse.dve_ops as dve_ops_mod
from concourse import bass_utils
from concourse._compat import with_exitstack
from concourse.dve_spec import (Spec, Src0, Src1, C0, C1, C2, Zero, select,
                                lower, _has_src1)
from concourse.dve_uop import DveOpSpec

FP32 = mybir.dt.float32
BF16 = mybir.dt.bfloat16
ALU = mybir.AluOpType
ActFn = mybir.ActivationFunctionType

B_FULL, NIN, T_FULL = 2048, 512, 100
H1, H2, O = 512, 256, 5
OP = 8
NCORES = 8
BS = B_FULL // NCORES


# ---------------- custom fused LIF op: u' = select(u<1,u,0)*c0 + g + c1 ----
def _lif2_reference(in0, in1, s0, s1, imm2):
    r = np.where(in0.astype(np.float32) < np.float32(imm2),
                 in0.astype(np.float32), np.float32(0.0)).astype(np.float32)
    return ((r * np.float32(s0) + in1.astype(np.float32)).astype(np.float32)
            + np.float32(s1)).astype(np.float32)


def _make_lif2_op():
    name = "LIF_STEP2_ANT"
    for o in dve_ops_mod.OPS:
        if o.name == name:
            return o
    body = select(Src0 < C2, Src0, Zero) * C0 + Src1 + C1
    spec = Spec(body=body, reference=_lif2_reference)
    shas = {}
    for ver in ("v3", "v4"):
        s = DveOpSpec(name=name, opcode=1, uops=lower(spec, ver=ver),
                      rd1_en=_has_src1(spec))
        shas[ver] = s.sha(ver)
    op = dve_ops_mod.DveOp(name, spec, subdim=False, uops_sha=shas)
    dve_ops_mod.OPS.append(op)
    dve_ops_mod.CUSTOM_DVE_SPECS[name] = spec
    dve_ops_mod._SUB_OPCODE_FOR_NAME[name] = (
        max(dve_ops_mod._SUB_OPCODE_FOR_NAME.values()) + 1)
    return op


LIF2_OP = _make_lif2_op()


def _build_snn(T: int = T_FULL, B: int = BS):
    nc = bacc.Bacc(trn_type="TRN2")
    x_d = nc.dram_tensor("x", [NIN, T, B], BF16, kind="ExternalInput")
    w1t_d = nc.dram_tensor("w1t", [NIN, H1], BF16, kind="ExternalInput")
    b1c_d = nc.dram_tensor("b1c", [H1, 1], FP32, kind="ExternalInput")
    w2t_d = nc.dram_tensor("w2t", [H1, H2], BF16, kind="ExternalInput")
    b2r_d = nc.dram_tensor("b2r", [1, H2], BF16, kind="ExternalInput")
    wot_d = nc.dram_tensor("wot", [H2, OP], BF16, kind="ExternalInput")
    boc_d = nc.dram_tensor("boc", [OP, 1], FP32, kind="ExternalInput")
    acc_d = nc.dram_tensor("acc", [OP, B], FP32, kind="ExternalOutput")

    with tile.TileContext(nc) as tc:
        _snn_body(tc, x_d, w1t_d, b1c_d, w2t_d, b2r_d, wot_d, boc_d, acc_d, T, B)
    nc.compile()
    return nc


@with_exitstack
def _snn_body(ctx: ExitStack, tc, x_d, w1t_d, b1c_d, w2t_d, b2r_d, wot_d,
              boc_d, acc_d, T, B):
    nc = tc.nc
    K1, M1 = 4, 4
    K2, M2 = 4, 2
    K3 = 2

    consts = ctx.enter_context(tc.tile_pool(name="consts", bufs=1))
    xin = ctx.enter_context(tc.tile_pool(name="xin", bufs=4))
    state = ctx.enter_context(tc.tile_pool(name="state", bufs=1))
    masks = ctx.enter_context(tc.tile_pool(name="masks", bufs=2))
    psum_g1 = ctx.enter_context(tc.tile_pool(name="psum_g1", bufs=2, space="PSUM"))
    psum_h2 = ctx.enter_context(tc.tile_pool(name="psum_h2", bufs=2, space="PSUM"))
    psum_h3 = ctx.enter_context(tc.tile_pool(name="psum_h3", bufs=2, space="PSUM"))

    w1_sb = []
    for k in range(K1):
        t_ = consts.tile([128, H1], BF16, tag=f"w1_{k}")
        nc.sync.dma_start(t_[:], w1t_d[k * 128:(k + 1) * 128, :])
        w1_sb.append(t_)
    w2_sb = []
    for k in range(K2):
        t_ = consts.tile([128, H2], BF16, tag=f"w2_{k}")
        nc.sync.dma_start(t_[:], w2t_d[k * 128:(k + 1) * 128, :])
        w2_sb.append(t_)
    wo_sb = []
    for k in range(K3):
        t_ = consts.tile([128, OP], BF16, tag=f"wo_{k}")
        nc.sync.dma_start(t_[:], wot_d[k * 128:(k + 1) * 128, :])
        wo_sb.append(t_)
    b1c = []
    for m in range(M1):
        t_ = consts.tile([128, 1], FP32, tag=f"b1c_{m}")
        nc.sync.dma_start(t_[:], b1c_d[m * 128:(m + 1) * 128, :])
        b1c.append(t_)
    b2r = consts.tile([1, H2], BF16, tag="b2r")
    nc.sync.dma_start(b2r[:], b2r_d[:, :])
    ones = consts.tile([1, B], BF16, tag="ones")
    nc.vector.memset(ones[:], 1.0)
    boc = consts.tile([OP, 1], FP32, tag="boc")
    nc.sync.dma_start(boc[:], boc_d[:, :])
    neg1 = consts.tile([128, 1], FP32, tag="neg1")
    nc.vector.memset(neg1[:], -1.0)

    u1 = state.tile([128, M1 * B], FP32, tag="u1")
    u2 = state.tile([128, M2 * B], FP32, tag="u2")
    u3 = state.tile([OP, B], FP32, tag="u3")
    acc = state.tile([OP, B], FP32, tag="acc")
    nc.vector.memset(u1[:], 0.0)
    nc.vector.memset(u2[:], 0.0)
    nc.vector.memset(u3[:], 0.0)
    nc.vector.memset(acc[:], 0.0)

    xpair_cache = {}

    def load_x(t):
        # DMA two timesteps at once; slice per step.
        t0 = t - (t % 2)
        if t0 not in xpair_cache:
            pair = []
            tw = min(2, T - t0)
            for k in range(K1):
                xt = xin.tile([128, tw * B], BF16, tag=f"x_{k}")
                nc.sync.dma_start(
                    xt[:].rearrange("p (tt b) -> p tt b", tt=tw),
                    x_d[k * 128:(k + 1) * 128, t0:t0 + tw, :])
                pair.append(xt)
            xpair_cache.clear()
            xpair_cache[t0] = pair
        off = (t - t0) * B
        return [xt[:, off:off + B] for xt in xpair_cache[t0]]

    def phase_a(t, x_tiles):
        g1 = psum_g1.tile([128, M1 * B], FP32, tag="g1")
        for m in range(M1):
            out = g1[:, m * B:(m + 1) * B]
            for k in range(K1):
                nc.tensor.matmul(out, w1_sb[k][:, m * 128:(m + 1) * 128],
                                 x_tiles[k][:], start=(k == 0), stop=(k == K1 - 1))
        return g1

    # pipeline registers: g1 slot, n1/n2 mask tiles
    g1_cur = phase_a(0, load_x(0))
    n1_cur = None     # n1 of step t-1 (input to L2)
    n2_cur = None     # n2 of step t-2 (input to L3)

    # Each iteration t: L1 advances to step t; L2 to t-1; L3 to t-2.
    for t in range(T + 2):
        # ---------------- PE work first (all inputs from earlier iters) ----
        if t + 1 < T:
            g1_next = phase_a(t + 1, load_x(t + 1))
        if t >= 1 and t - 1 < T:
            # layer 2 matmul for step t-1 using n1(t-1)
            h2 = psum_h2.tile([128, M2 * B], FP32, tag="h2")
            for m in range(M2):
                out = h2[:, m * B:(m + 1) * B]
                nc.tensor.matmul(out, b2r[0:1, m * 128:(m + 1) * 128],
                                 ones[0:1, :], start=True, stop=False)
                for k in range(K2):
                    nc.tensor.matmul(out, w2_sb[k][:, m * 128:(m + 1) * 128],
                                     n1_cur[:, k * B:(k + 1) * B],
                                     start=False, stop=(k == K2 - 1))
        if t >= 2 and t - 2 < T:
            h3 = psum_h3.tile([OP, B], FP32, tag="h3")
            for k in range(K3):
                nc.tensor.matmul(h3[:, :], wo_sb[k][:, :],
                                 n2_cur[:, k * B:(k + 1) * B],
                                 start=(k == 0), stop=(k == K3 - 1))

        # ---------------- layer 1: LIF + spike sign ----
        if t < T:
            for m in range(M1):
                nc.vector._custom_dve(
                    LIF2_OP, out=u1[:, m * B:(m + 1) * B],
                    in0=u1[:, m * B:(m + 1) * B], in1=g1_cur[:, m * B:(m + 1) * B],
                    s0=0.5, s1=b1c[m][:, :], imm2=1.0)
            if t + 1 < T:
                g1_cur = g1_next
            n1_new = masks.tile([128, M1 * B], BF16, tag="n1")
            nc.scalar.sign(n1_new[:], u1[:], bias=neg1[:, :])
            n1_cur = n1_new

        # ---------------- layer 2: LIF + spike sign (step t-1) ----
        if t >= 1 and t - 1 < T:
            nc.vector._custom_dve(LIF2_OP, out=u2[:], in0=u2[:], in1=h2[:],
                                  s0=0.5, s1=0.0, imm2=1.0)
            n2_new = masks.tile([128, M2 * B], BF16, tag="n2")
            nc.scalar.sign(n2_new[:], u2[:], bias=neg1[:, :])
            n2_cur = n2_new

        # ---------------- layer 3: LIF + accumulate (step t-2) ----
        if t >= 2 and t - 2 < T:
            nc.vector._custom_dve(LIF2_OP, out=u3[:], in0=u3[:], in1=h3[:],
                                  s0=0.5, s1=boc[:, :], imm2=1.0)
            nc.vector.scalar_tensor_tensor(acc[:, :], u3[:, :], 1.0, acc[:, :],
                                           op0=ALU.is_ge, op1=ALU.add)

    nc.sync.dma_start(acc_d[:, :], acc[:, :])


_NC_CACHE = {}


def _get_nc(T, B):
    key = (T, B)
    if key not in _NC_CACHE:
        _NC_CACHE[key] = _build_snn(T, B)
    return _NC_CACHE[key]


def _prep_in_maps(x, W1, b1, W2, b2, Wo, bo):
    bf = ml_dtypes.bfloat16
    W1 = W1.astype(np.float32); b1 = b1.astype(np.float32)
    W2 = W2.astype(np.float32); b2 = b2.astype(np.float32)
    Wo = Wo.astype(np.float32); bo = bo.astype(np.float32)
    # layer-2/3 folds: s = (n+1)/2  ->  g = 0.25*(n@W.T) + 0.25*rowsum(W) + 0.5*b
    b2f = 0.25 * W2.sum(axis=1) + 0.5 * b2
    bof = 0.25 * Wo.sum(axis=1) + 0.5 * bo
    base = {
        "w1t": np.ascontiguousarray((0.5 * W1).T).astype(bf),
        "b1c": (0.5 * b1).reshape(-1, 1).astype(np.float32),
        "w2t": np.ascontiguousarray((0.25 * W2).T).astype(bf),
        "b2r": b2f.reshape(1, -1).astype(bf),
        "wot": np.ascontiguousarray(
            np.pad((0.25 * Wo).T, ((0, 0), (0, OP - O)))).astype(bf),
        "boc": np.pad(bof, (0, OP - O)).reshape(-1, 1).astype(np.float32),
    }
    in_maps = []
    for c in range(NCORES):
        xs = x[c * BS:(c + 1) * BS]
        xs = np.ascontiguousarray(xs.transpose(1, 2, 0)).astype(bf)
        in_maps.append({**base, "x": xs})
    return in_maps


def kernel(x, W1, b1, W2, b2, Wo, bo, _trace=False, _trace_kwargs=None):
    x = np.asarray(x)
    T = x.shape[2]
    nc = _get_nc(T, BS)
    in_maps = _prep_in_maps(np.asarray(x, np.float32), np.asarray(W1),
                            np.asarray(b1), np.asarray(W2), np.asarray(b2),
                            np.asarray(Wo), np.asarray(bo))
    kw = {}
    if _trace:
        kw = {"trace": True, **(_trace_kwargs or {})}
    r = bass_utils.run_bass_kernel_spmd(nc, in_maps, core_ids=list(range(NCORES)), **kw)
    outs = []
    for c in range(NCORES):
        accv = r.results[c]["acc"]                        # (OP, BS) = sum of n3
        outs.append(accv[:O, :].T / np.float32(T))
    out = np.concatenate(outs, axis=0).astype(np.float32)
    kernel._last_results = r
    return out


# revision 9
# speedup vs baseline: 1.3991x; 1.3991x over previous
"""Trainium2 Bass kernel for nn_PopulationSNN: 3-layer LIF SNN, T=100 timesteps.

Sharding: data-parallel over batch — 2048 rows split across 8 NeuronCores
(256 rows each); weights replicated; the sequential timestep scan runs
independently per shard.

Per-core structure, software-pipelined across layers (layer L runs one
timestep behind layer L-1):
  phase A : g1[t] = x_t @ (0.5*W1).T + 0.5*b1 on TensorE (bf16 operands,
            fp32 PSUM accumulate; bias via a K=1 ones-matmul).
  LIF     : state u = pre-reset membrane. One fused custom-DVE op:
               u' = select(u < 1, u, 0)*0.5 + g + bias
  spikes  : n = sign(u - 1) on ScalarE as bf16; the next layer's matmul
            consumes n directly with folded weights W'' = 0.25*W and
            bias'' = 0.25*rowsum(W) + 0.5*b   (since s = (n+1)/2).
  output  : acc += n3 on VectorE; host computes (acc + T) / (2T).
"""

from contextlib import ExitStack

import numpy as np
import ml_dtypes

import concourse.bacc as bacc
import concourse.mybir as mybir
import concourse.tile as tile
import concourse.dve_ops as dve_ops_mod
from concourse import bass_utils
from concourse._compat import with_exitstack
from concourse.dve_spec import (Spec, Src0, Src1, C0, C1, C2, Zero, select,
                                lower, _has_src1)
from concourse.dve_uop import DveOpSpec

FP32 = mybir.dt.float32
BF16 = mybir.dt.bfloat16
ALU = mybir.AluOpType
ActFn = mybir.ActivationFunctionType

B_FULL, NIN, T_FULL = 2048, 512, 100
H1, H2, O = 512, 256, 5
OP = 8
NCORES = 8
BS = B_FULL // NCORES


# ---------------- custom fused LIF op: u' = select(u<1,u,0)*c0 + g + c1 ----
def _lif2_reference(in0, in1, s0, s1, imm2):
    r = np.where(in0.astype(np.float32) < np.float32(imm2),
                 in0.astype(np.float32), np.float32(0.0)).astype(np.float32)
    return ((r * np.float32(s0) + in1.astype(np.float32)).astype(np.float32)
            + np.float32(s1)).astype(np.float32)


def _make_lif2_op():
    name = "LIF_STEP2_ANT"
    for o in dve_ops_mod.OPS:
        if o.name == name:
            return o
    body = select(Src0 < C2, Src0, Zero) * C0 + Src1 + C1
    spec = Spec(body=body, reference=_lif2_reference)
    shas = {}
    for ver in ("v3", "v4"):
        s = DveOpSpec(name=name, opcode=1, uops=lower(spec, ver=ver),
                      rd1_en=_has_src1(spec))
        shas[ver] = s.sha(ver)
    op = dve_ops_mod.DveOp(name, spec, subdim=False, uops_sha=shas)
    dve_ops_mod.OPS.append(op)
    dve_ops_mod.CUSTOM_DVE_SPECS[name] = spec
    dve_ops_mod._SUB_OPCODE_FOR_NAME[name] = (
        max(dve_ops_mod._SUB_OPCODE_FOR_NAME.values()) + 1)
    return op


LIF2_OP = _make_lif2_op()


def _build_snn(T: int = T_FULL, B: int = BS):
    nc = bacc.Bacc(trn_type="TRN2")
    x_d = nc.dram_tensor("x", [NIN, T, B], BF16, kind="ExternalInput")
    w1t_d = nc.dram_tensor("w1t", [NIN, H1], BF16, kind="ExternalInput")
    b1c_d = nc.dram_tensor("b1c", [H1, 1], FP32, kind="ExternalInput")
    w2t_d = nc.dram_tensor("w2t", [H1, H2], BF16, kind="ExternalInput")
    b2c_d = nc.dram_tensor("b2c", [H2, 1], FP32, kind="ExternalInput")
    wot_d = nc.dram_tensor("wot", [H2, OP], BF16, kind="ExternalInput")
    boc_d = nc.dram_tensor("boc", [OP, 1], FP32, kind="ExternalInput")
    acc_d = nc.dram_tensor("acc", [OP, B], FP32, kind="ExternalOutput")

    with tile.TileContext(nc) as tc:
        _snn_body(tc, x_d, w1t_d, b1c_d, w2t_d, b2c_d, wot_d, boc_d, acc_d, T, B)
    nc.compile()
    return nc


@with_exitstack
def _snn_body(ctx: ExitStack, tc, x_d, w1t_d, b1c_d, w2t_d, b2c_d, wot_d,
              boc_d, acc_d, T, B):
    nc = tc.nc
    K1, M1 = 4, 4
    K2, M2 = 4, 2
    K3 = 2

    consts = ctx.enter_context(tc.tile_pool(name="consts", bufs=1))
    xin = ctx.enter_context(tc.tile_pool(name="xin", bufs=4))
    state = ctx.enter_context(tc.tile_pool(name="state", bufs=1))
    masks = ctx.enter_context(tc.tile_pool(name="masks", bufs=2))
    psum_g1 = ctx.enter_context(tc.tile_pool(name="psum_g1", bufs=2, space="PSUM"))
    psum_h2 = ctx.enter_context(tc.tile_pool(name="psum_h2", bufs=2, space="PSUM"))
    psum_h3 = ctx.enter_context(tc.tile_pool(name="psum_h3", bufs=2, space="PSUM"))

    w1_sb = []
    for k in range(K1):
        t_ = consts.tile([128, H1], BF16, tag=f"w1_{k}")
        nc.sync.dma_start(t_[:], w1t_d[k * 128:(k + 1) * 128, :])
        w1_sb.append(t_)
    w2_sb = []
    for k in range(K2):
        t_ = consts.tile([128, H2], BF16, tag=f"w2_{k}")
        nc.sync.dma_start(t_[:], w2t_d[k * 128:(k + 1) * 128, :])
        w2_sb.append(t_)
    wo_sb = []
    for k in range(K3):
        t_ = consts.tile([128, OP], BF16, tag=f"wo_{k}")
        nc.sync.dma_start(t_[:], wot_d[k * 128:(k + 1) * 128, :])
        wo_sb.append(t_)
    b1c = []
    for m in range(M1):
        t_ = consts.tile([128, 1], FP32, tag=f"b1c_{m}")
        nc.sync.dma_start(t_[:], b1c_d[m * 128:(m + 1) * 128, :])
        b1c.append(t_)
    b2c = []
    for m in range(M2):
        t_ = consts.tile([128, 1], FP32, tag=f"b2c_{m}")
        nc.sync.dma_start(t_[:], b2c_d[m * 128:(m + 1) * 128, :])
        b2c.append(t_)
    boc = consts.tile([OP, 1], FP32, tag="boc")
    nc.sync.dma_start(boc[:], boc_d[:, :])
    neg1 = consts.tile([128, 1], FP32, tag="neg1")
    nc.vector.memset(neg1[:], -1.0)

    u1 = state.tile([128, M1 * B], FP32, tag="u1")
    u2 = state.tile([128, M2 * B], FP32, tag="u2")
    u3 = state.tile([OP, B], FP32, tag="u3")
    acc = state.tile([OP, B], FP32, tag="acc")
    nc.vector.memset(u1[:], 0.0)
    nc.vector.memset(u2[:], 0.0)
    nc.vector.memset(u3[:], 0.0)
    nc.vector.memset(acc[:], 0.0)

    xpair_cache = {}

    def load_x(t):
        # DMA two timesteps at once; slice per step.
        t0 = t - (t % 2)
        if t0 not in xpair_cache:
            pair = []
            tw = min(2, T - t0)
            for k in range(K1):
                xt = xin.tile([128, tw * B], BF16, tag=f"x_{k}")
                nc.sync.dma_start(
                    xt[:].rearrange("p (tt b) -> p tt b", tt=tw),
                    x_d[k * 128:(k + 1) * 128, t0:t0 + tw, :])
                pair.append(xt)
            xpair_cache.clear()
            xpair_cache[t0] = pair
        off = (t - t0) * B
        return [xt[:, off:off + B] for xt in xpair_cache[t0]]

    def phase_a(t, x_tiles):
        g1 = psum_g1.tile([128, M1 * B], FP32, tag="g1")
        for m in range(M1):
            out = g1[:, m * B:(m + 1) * B]
            for k in range(K1):
                nc.tensor.matmul(out, w1_sb[k][:, m * 128:(m + 1) * 128],
                                 x_tiles[k][:], start=(k == 0), stop=(k == K1 - 1))
        return g1

    # pipeline registers: g1 slot, n1/n2 mask tiles
    g1_cur = phase_a(0, load_x(0))
    n1_cur = None     # n1 of step t-1 (input to L2)
    n2_cur = None     # n2 of step t-2 (input to L3)

    # Each iteration t: L1 advances to step t; L2 to t-1; L3 to t-2.
    for t in range(T + 2):
        # ---------------- PE work first (all inputs from earlier iters) ----
        if t + 1 < T:
            g1_next = phase_a(t + 1, load_x(t + 1))
        if t >= 1 and t - 1 < T:
            # layer 2 matmul for step t-1 using n1(t-1)
            h2 = psum_h2.tile([128, M2 * B], FP32, tag="h2")
            for m in range(M2):
                out = h2[:, m * B:(m + 1) * B]
                for k in range(K2):
                    nc.tensor.matmul(out, w2_sb[k][:, m * 128:(m + 1) * 128],
                                     n1_cur[:, k * B:(k + 1) * B],
                                     start=(k == 0), stop=(k == K2 - 1))
        if t >= 2 and t - 2 < T:
            h3 = psum_h3.tile([OP, B], FP32, tag="h3")
            for k in range(K3):
                nc.tensor.matmul(h3[:, :], wo_sb[k][:, :],
                                 n2_cur[:, k * B:(k + 1) * B],
                                 start=(k == 0), stop=(k == K3 - 1))

        # ---------------- layer 1: LIF + spike sign ----
        if t < T:
            for m in range(M1):
                nc.vector._custom_dve(
                    LIF2_OP, out=u1[:, m * B:(m + 1) * B],
                    in0=u1[:, m * B:(m + 1) * B], in1=g1_cur[:, m * B:(m + 1) * B],
                    s0=0.5, s1=b1c[m][:, :], imm2=1.0)
            if t + 1 < T:
                g1_cur = g1_next
            n1_new = masks.tile([128, M1 * B], BF16, tag="n1")
            nc.scalar.sign(n1_new[:], u1[:], bias=neg1[:, :])
            n1_cur = n1_new

        # ---------------- layer 2: LIF + spike sign (step t-1) ----
        if t >= 1 and t - 1 < T:
            for m in range(M2):
                nc.vector._custom_dve(
                    LIF2_OP, out=u2[:, m * B:(m + 1) * B],
                    in0=u2[:, m * B:(m + 1) * B], in1=h2[:, m * B:(m + 1) * B],
                    s0=0.5, s1=b2c[m][:, :], imm2=1.0)
            n2_new = masks.tile([128, M2 * B], BF16, tag="n2")
            nc.scalar.sign(n2_new[:], u2[:], bias=neg1[:, :])
            n2_cur = n2_new

        # ---------------- layer 3: LIF + accumulate (step t-2) ----
        if t >= 2 and t - 2 < T:
            nc.vector._custom_dve(LIF2_OP, out=u3[:], in0=u3[:], in1=h3[:],
                                  s0=0.5, s1=boc[:, :], imm2=1.0)
            nc.vector.scalar_tensor_tensor(acc[:, :], u3[:, :], 1.0, acc[:, :],
                                           op0=ALU.is_ge, op1=ALU.add)

    nc.sync.dma_start(acc_d[:, :], acc[:, :])


_NC_CACHE = {}


def _get_nc(T, B):
    key = (T, B)
    if key not in _NC_CACHE:
        _NC_CACHE[key] = _build_snn(T, B)
    return _NC_CACHE[key]


def _prep_in_maps(x, W1, b1, W2, b2, Wo, bo):
    bf = ml_dtypes.bfloat16
    W1 = W1.astype(np.float32); b1 = b1.astype(np.float32)
    W2 = W2.astype(np.float32); b2 = b2.astype(np.float32)
    Wo = Wo.astype(np.float32); bo = bo.astype(np.float32)
    # layer-2/3 folds: s = (n+1)/2  ->  g = 0.25*(n@W.T) + 0.25*rowsum(W) + 0.5*b
    b2f = 0.25 * W2.sum(axis=1) + 0.5 * b2
    bof = 0.25 * Wo.sum(axis=1) + 0.5 * bo
    base = {
        "w1t": np.ascontiguousarray((0.5 * W1).T).astype(bf),
        "b1c": (0.5 * b1).reshape(-1, 1).astype(np.float32),
        "w2t": np.ascontiguousarray((0.25 * W2).T).astype(bf),
        "b2c": b2f.reshape(-1, 1).astype(np.float32),
        "wot": np.ascontiguousarray(
            np.pad((0.25 * Wo).T, ((0, 0), (0, OP - O)))).astype(bf),
        "boc": np.pad(bof, (0, OP - O)).reshape(-1, 1).astype(np.float32),
    }
    in_maps = []
    for c in range(NCORES):
        xs = x[c * BS:(c + 1) * BS]
        xs = np.ascontiguousarray(xs.transpose(1, 2, 0)).astype(bf)
        in_maps.append({**base, "x": xs})
    return in_maps


def kernel(x, W1, b1, W2, b2, Wo, bo, _trace=False, _trace_kwargs=None):
    x = np.asarray(x)
    T = x.shape[2]
    nc = _get_nc(T, BS)
    in_maps = _prep_in_maps(np.asarray(x, np.float32), np.asarray(W1),
                            np.asarray(b1), np.asarray(W2), np.asarray(b2),
                            np.asarray(Wo), np.asarray(bo))
    kw = {}
    if _trace:
        kw = {"trace": True, **(_trace_kwargs or {})}
    r = bass_utils.run_bass_kernel_spmd(nc, in_maps, core_ids=list(range(NCORES)), **kw)
    outs = []
    for c in range(NCORES):
        accv = r.results[c]["acc"]                        # (OP, BS) = sum of n3
        outs.append(accv[:O, :].T / np.float32(T))
    out = np.concatenate(outs, axis=0).astype(np.float32)
    kernel._last_results = r
    return out
